# revision 1
# baseline (speedup 1.0000x reference)
"""Trainium2 Bass kernel for GaussianKernelLayer.

y[n] = sum_m softmax(coef)[m] * norm * exp(-0.5*|x_n - c_m|^2),
N=500000, M=256, D=4, sigma=1. Data-parallel over 8 cores (x sharded on N).

Device strategy (per core, NP=63488 padded rows, 124 chunks of 512):
  - K=16 fp16 matmul computes the FULL exp argument in PSUM:
      psum[m, n] = x.c (hi/lo split) + [ln(norm*w_m) - 0.5|c_m|^2] - 0.5|x_n|^2
    centers live on PSUM partitions (2 halves of 128), x streams as rhs.
  - -0.5|x|^2 is computed on-device (DVE square+reduce in a blocked layout),
    split hi/lo to fp16, bounced through a DRAM scratch so it can be DMA'd
    into rhs rows 14-15 in the streaming layout.
  - ACT does one big Exp per chunk: [128, 1024] PSUM -> fp16 SBUF.
  - DVE adds the two center-halves; PE reduces 128 partitions with a ones
    matmul (M=32 so a 4-chunk group fills all 128 partitions of one PSUM
    bank); DVE evacuates, DMA writes y.
"""

import math

import numpy as np

import concourse.bass as bass
import concourse.bacc as bacc_mod
import concourse.mybir as mybir
from concourse.bass_utils import run_bass_kernel_spmd
from concourse.tile import TileContext

N_CORES = 8
N_TOTAL = 500000
PER_CORE = N_TOTAL // N_CORES  # 62500
CHUNK = 512
NCHUNK = 124
NP = CHUNK * NCHUNK  # 63488 = 128 * 496
R = NP // 128  # 496
M = 256
D = 4
SIGMA = 1.0

F16 = mybir.dt.float16
F32 = mybir.dt.float32

_CACHE = {}


def _build_nc():
    nc = bacc_mod.Bacc()

    rhs_d = nc.dram_tensor("rhs", [14, NP], F16, kind="ExternalInput")
    xnat_d = nc.dram_tensor("xnat", [128, 4 * R], F32, kind="ExternalInput")
    lhsT_d = nc.dram_tensor("lhsT", [16, 256], F16, kind="ExternalInput")
    y_d = nc.dram_tensor("y", [NP], F32, kind="ExternalOutput")
    biasrow_d = nc.dram_tensor("biasrow", [2, NP], F16)  # internal scratch

    with TileContext(nc) as tc:
        with (
            tc.tile_pool(name="const", bufs=1) as constp,
            tc.tile_pool(name="pre", bufs=1) as prep,
            tc.tile_pool(name="rhsp", bufs=3) as rhsp,
            tc.tile_pool(name="expp", bufs=5) as expp,
            tc.tile_pool(name="combp", bufs=4) as combp,
            tc.tile_pool(name="ycp", bufs=3) as ycp,
            tc.tile_pool(name="psp", bufs=3, space="PSUM") as psp,
            tc.tile_pool(name="redp", bufs=2, space="PSUM") as redp,
        ):
            # --- constants ---
            lhsT_sb = constp.tile([16, 256], F16)
            nc.sync.dma_start(lhsT_sb[:], lhsT_d[:])
            ones_red = constp.tile([128, 32], F16)
            nc.vector.memset(ones_red[:], 1.0)

            # --- preamble: bias rows = -0.5*|x|^2 in fp16 hi/lo ---
            xn = prep.tile([128, 4 * R], F32)
            nc.sync.dma_start(xn[:], xnat_d[:])
            sq = prep.tile([128, 4 * R], F32)
            nc.vector.tensor_tensor(sq[:], xn[:], xn[:], mybir.AluOpType.mult)
            s = prep.tile([128, R], F32)
            nc.vector.tensor_reduce(
                s[:],
                sq[:].rearrange("p (f d) -> p f d", d=4),
                axis=mybir.AxisListType.X,
                op=mybir.AluOpType.add,
            )
            sh = prep.tile([128, R], F32)
            nc.vector.tensor_scalar_mul(sh[:], s[:], -0.5)
            bp = prep.tile([128, 2 * R], F16)
            nc.vector.tensor_copy(bp[:, 0:R], sh[:])
            # (bias_hi * -1) + sh = sh - bias_hi
            nc.vector.scalar_tensor_tensor(
                bp[:, R : 2 * R],
                bp[:, 0:R],
                -1.0,
                sh[:],
                mybir.AluOpType.mult,
                mybir.AluOpType.add,
            )
            # funnel in 4 partition-quarters so early chunks only wait on the
            # first quarter: partitions 32q..32q+32 hold n in [q*NP/4, ...)
            NQ = NP // 4
            for fq in range(4):
                nc.sync.dma_start(
                    biasrow_d[:, fq * NQ : (fq + 1) * NQ].rearrange(
                        "t (p f) -> p t f", p=32
                    ),
                    bp[32 * fq : 32 * fq + 32, :].rearrange("p (t f) -> p t f", t=2),
                )

            # --- main loop: groups of G chunks share one rhs DMA pair ---
            G = 8
            rp = None
            for g0 in range(0, NCHUNK, G):
                gsz = min(G, NCHUNK - g0)
                rhs_t = rhsp.tile([16, G * CHUNK], F16, tag="rhs")
                nc.sync.dma_start(
                    rhs_t[0:14, 0 : gsz * CHUNK],
                    rhs_d[:, g0 * CHUNK : (g0 + gsz) * CHUNK],
                )
                nc.sync.dma_start(
                    rhs_t[14:16, 0 : gsz * CHUNK],
                    biasrow_d[:, g0 * CHUNK : (g0 + gsz) * CHUNK],
                )
                for kk in range(gsz):
                    k = g0 + kk
                    rcol = kk * CHUNK
                    ps = psp.tile([128, 2 * CHUNK], F32, tag="ps")
                    nc.tensor.matmul(
                        ps[:, 0:CHUNK],
                        lhsT_sb[:, 0:128],
                        rhs_t[:, rcol : rcol + CHUNK],
                        start=True,
                        stop=True,
                    )
                    nc.tensor.matmul(
                        ps[:, CHUNK : 2 * CHUNK],
                        lhsT_sb[:, 128:256],
                        rhs_t[:, rcol : rcol + CHUNK],
                        start=True,
                        stop=True,
                    )

                    ex = expp.tile([128, 2 * CHUNK], F16, tag="ex")
                    nc.scalar.activation(
                        ex[:], ps[:], mybir.ActivationFunctionType.Exp
                    )

                    cb = combp.tile([128, CHUNK], F16, tag="cb")
                    nc.vector.tensor_tensor(
                        cb[:], ex[:, 0:CHUNK], ex[:, CHUNK : 2 * CHUNK],
                        mybir.AluOpType.add,
                    )

                    q = k % 4
                    if q == 0:
                        rp = redp.tile([128, CHUNK], F32, tag="rp")
                    nc.tensor.matmul(
                        rp[32 * q : 32 * q + 32, :],
                        ones_red[:],
                        cb[:],
                        start=True,
                        stop=True,
                        tile_position=(0, 32 * q),
                    )

                    if q == 3:
                        j = k // 4
                        yc = ycp.tile([128, CHUNK], F32, tag="yc")
                        nc.vector.tensor_copy(yc[:], rp[:])
                        nc.sync.dma_start(
                            y_d[4 * j * CHUNK : (4 * j + 4) * CHUNK].rearrange(
                                "(p f) -> p f", p=4
                            ),
                            yc[0:97:32, :],
                        )
    nc.compile()
    return nc


def _host_prep(x, centers, coefficients):
    """Small host-side prep: softmax over 256 coefficients, center hi/lo
    split, per-center bias. All O(M) except the per-core x layout work."""
    x = np.ascontiguousarray(np.asarray(x, dtype=np.float32))
    centers = np.asarray(centers, dtype=np.float32)
    coefficients = np.asarray(coefficients, dtype=np.float32)

    norm_const = np.float32(1.0 / ((2.0 * math.pi) ** (D / 2) * SIGMA**D))
    e = np.exp(coefficients - coefficients.max())
    w = (e / e.sum()).astype(np.float32)
    b = np.log(w * norm_const).astype(np.float32) - 0.5 * (centers**2).sum(axis=1)

    cT = centers.T  # [4, 256]
    c_hi = cT.astype(np.float16)
    c_lo = (cT - c_hi.astype(np.float32)).astype(np.float16)
    b_hi = b.astype(np.float16)
    b_lo = (b - b_hi.astype(np.float32)).astype(np.float16)

    lhsT = np.empty((16, 256), dtype=np.float16)
    lhsT[0:4] = c_hi
    lhsT[4:8] = c_hi
    lhsT[8:12] = c_lo
    lhsT[12] = b_hi
    lhsT[13] = b_lo
    lhsT[14] = 1.0
    lhsT[15] = 1.0

    in_maps = []
    for i in range(N_CORES):
        xs = x[i * PER_CORE : (i + 1) * PER_CORE]
        xp = np.zeros((NP, D), dtype=np.float32)
        xp[:PER_CORE] = xs
        xh = xp.astype(np.float16)
        xl = (xp - xh.astype(np.float32)).astype(np.float16)
        rhs = np.empty((14, NP), dtype=np.float16)
        rhs[0:4] = xh.T
        rhs[4:8] = xl.T
        rhs[8:12] = xh.T
        rhs[12] = 1.0
        rhs[13] = 1.0
        xnat = np.ascontiguousarray(xp.reshape(128, R * D))
        in_maps.append({"rhs": rhs, "xnat": xnat, "lhsT": lhsT.copy()})
    return in_maps


last_result = None


def kernel(x, centers, coefficients):
    global last_result
    if "nc" not in _CACHE:
        _CACHE["nc"] = _build_nc()
    nc = _CACHE["nc"]
    in_maps = _host_prep(x, centers, coefficients)
    res = run_bass_kernel_spmd(nc, in_maps, core_ids=list(range(N_CORES)))
    last_result = res
    y = np.concatenate([r["y"][:PER_CORE] for r in res.results])
    return y.astype(np.float32)



# revision 6
# speedup vs baseline: 1.4350x; 1.4350x over previous
"""Trainium2 Bass kernel for GaussianKernelLayer.

y[n] = sum_m softmax(coef)[m] * norm * exp(-0.5*|x_n - c_m|^2),
N=500000, M=256, D=4, sigma=1. Data-parallel over 8 cores (x sharded on N).

Device strategy (per core, NP=63488 padded points = 496 lanes... see below):

The exp work on the Scalar (ACT) engine is the hard floor: N*M/core =
16.25M elements at 1 elem/cycle/partition @ 1.2 GHz ~= 104 us. Everything
else is shaped to hide under that:

  - [point, center] layout: psum[pt, 256*a + ctr] holds the full exp
    argument z = x.c + ln(w*norm) - 0.5|c|^2 - 0.5|x|^2 for 8 point-blocks
    (a = 0..7) at once. Stationary = x-features [K=128, 128 pts] with the
    8 blocks STACKED along K (16 rows each: x_hi(4) x_hi(4) x_lo(4)
    sq_hi sq_lo 1 1); moving = a constant block-diagonal center matrix
    [128, 2048] so one 2048-column matmul computes 1024 points * 256
    centers (2 PE cycles/point, one instruction per 1024 points).
  - ACT does one big Exp per group: [128, 2048] PSUM f32 -> SBUF fp16.
  - DVE does a segmented reduce over centers: [128, 8, 256] -> [128, 8]
    fp16 (2x mode), writing y for 1024 points directly. No reduction
    matmuls, no partition reductions, no add passes.
  - All per-center and per-point bias terms are folded on the host into
    the fp16 hi/lo split streams; no device-side preamble, no DRAM
    bounce. DMA in ~2.5 MB, out 254 KB.

Point -> (lane, slot) mapping n = m*496 + (8g + a) keeps the final y DMA
partition-major (1984 B contiguous per partition).
"""

import math

import numpy as np

import concourse.bass as bass
import concourse.bacc as bacc_mod
import concourse.mybir as mybir
from concourse.bass_utils import run_bass_kernel_spmd
from concourse.tile import TileContext

N_CORES = 8
N_TOTAL = 500000
PER_CORE = N_TOTAL // N_CORES  # 62500
M = 256
D = 4
SIGMA = 1.0

NG = 62            # groups per core
BLK = 8            # point-blocks per group (stacked along K)
GPTS = 128 * BLK   # 1024 points per group
NP = NG * GPTS     # 63488 padded points per core
SLOTS = NP // 128  # 496 slots per lane
XCOLS = NG * 128   # 7936 stationary columns

F16 = mybir.dt.float16
F32 = mybir.dt.float32

_CACHE = {}


def _build_nc():
    nc = bacc_mod.Bacc()

    xs_d = nc.dram_tensor("xs", [128, XCOLS], F16, kind="ExternalInput")
    cd_d = nc.dram_tensor("cd", [128, 2 * M], F16, kind="ExternalInput")
    y_d = nc.dram_tensor("y", [NP], F32, kind="ExternalOutput")

    with TileContext(nc) as tc:
        with (
            tc.tile_pool(name="const", bufs=1) as constp,
            tc.tile_pool(name="xsp", bufs=8) as xsp,
            tc.tile_pool(name="expp", bufs=3) as expp,
            tc.tile_pool(name="yp", bufs=1) as yp,
            tc.tile_pool(name="psp", bufs=2, space="PSUM") as psp,
        ):
            cd_sb = constp.tile([128, 2 * M], F16)
            nc.sync.dma_start(cd_sb[:], cd_d[:])

            # stationary stream, 8 resident slabs of 8 groups each
            SLAB = 1024
            slabs = []
            for s in range(8):
                w = min(SLAB, XCOLS - s * SLAB)
                t = xsp.tile([128, SLAB], F16, tag="xs")
                nc.sync.dma_start(t[:, 0:w], xs_d[:, s * SLAB : s * SLAB + w])
                slabs.append(t)

            ys = yp.tile([128, SLOTS + 16], F16, tag="ys")

            for g in range(NG):
                s, off = divmod(g, 8)
                col = off * 128
                ps = psp.tile([128, BLK * M], F32, tag="ps")
                # ISA caps matmul moving free at 512: one matmul per pair of
                # point-blocks, PE 32-row tiles so the weight load is K=32.
                for a in range(4):
                    nc.tensor.matmul(
                        ps[:, 512 * a : 512 * (a + 1)],
                        slabs[s][32 * a : 32 * a + 32, col : col + 128],
                        cd_sb[32 * a : 32 * a + 32, :],
                        start=True,
                        stop=True,
                        tile_position=(32 * a, 0),
                    )
                ex = expp.tile([128, BLK * M], F16, tag="ex")
                nc.scalar.activation(
                    ex[:], ps[:], mybir.ActivationFunctionType.Exp
                )
                with nc.allow_low_precision(reason="fp16 y, rel tol 2e-2"):
                    nc.vector.tensor_reduce(
                        ys[:, BLK * g : BLK * (g + 1)],
                        ex[:].rearrange("p (a c) -> p a c", c=M),
                        axis=mybir.AxisListType.X,
                        op=mybir.AluOpType.add,
                    )

            yf = yp.tile([128, SLOTS], F32, tag="yf")
            nc.vector.tensor_copy(yf[:], ys[:, 0:SLOTS])
            nc.sync.dma_start(y_d.rearrange("(p f) -> p f", p=128), yf[:])
    nc.compile()
    return nc


def _host_prep(x, centers, coefficients):
    """Host-side prep: softmax over 256 coefficients, fp16 hi/lo splits,
    per-center and per-point bias folding, streaming layout."""
    x = np.ascontiguousarray(np.asarray(x, dtype=np.float32))
    centers = np.asarray(centers, dtype=np.float32)
    coefficients = np.asarray(coefficients, dtype=np.float32)

    norm_const = np.float32(1.0 / ((2.0 * math.pi) ** (D / 2) * SIGMA**D))
    e = np.exp(coefficients - coefficients.max())
    w = (e / e.sum()).astype(np.float32)
    b = np.log(w * norm_const).astype(np.float32) - 0.5 * (centers**2).sum(axis=1)

    cT = centers.T  # [4, 256]
    c_hi = cT.astype(np.float16)
    c_lo = (cT - c_hi.astype(np.float32)).astype(np.float16)
    b_hi = b.astype(np.float16)
    b_lo = (b - b_hi.astype(np.float32)).astype(np.float16)

    crows = np.empty((16, M), dtype=np.float16)
    crows[0:4] = c_hi
    crows[4:8] = c_lo
    crows[8:12] = c_hi
    crows[12:14] = 1.0
    crows[14] = b_hi
    crows[15] = b_lo

    # [32, 512] two-block diagonal, replicated on all four 32-row bands so
    # band q's slice pairs with stationary rows 32q:32q+32 (blocks 2q, 2q+1)
    cd = np.zeros((128, 2 * M), dtype=np.float16)
    for q in range(4):
        cd[32 * q : 32 * q + 16, 0:M] = crows
        cd[32 * q + 16 : 32 * q + 32, M : 2 * M] = crows

    in_maps = []
    for i in range(N_CORES):
        xs = x[i * PER_CORE : (i + 1) * PER_CORE]
        xp = np.zeros((NP, D), dtype=np.float32)
        xp[:PER_CORE] = xs
        xh = xp.astype(np.float16)
        xl = (xp - xh.astype(np.float32)).astype(np.float16)
        sq = -0.5 * (xp * xp).sum(axis=1)
        sq_hi = sq.astype(np.float16)
        sq_lo = (sq - sq_hi.astype(np.float32)).astype(np.float16)

        feat = np.empty((16, NP), dtype=np.float16)
        feat[0:4] = xh.T      # pairs with c_hi
        feat[4:8] = xh.T      # pairs with c_lo
        feat[8:12] = xl.T     # pairs with c_hi
        feat[12] = sq_hi      # pairs with 1
        feat[13] = sq_lo      # pairs with 1
        feat[14:16] = 1.0     # pairs with b_hi / b_lo

        # n = m*496 + 8g + a  ->  xs_d[16a + k, g*128 + m] = feat[k, n]
        xsd = (
            feat.reshape(16, 128, NG, BLK)
            .transpose(3, 0, 2, 1)
            .reshape(128, XCOLS)
        )
        in_maps.append(
            {"xs": np.ascontiguousarray(xsd), "cd": cd.copy()}
        )
    return in_maps


last_result = None


def kernel(x, centers, coefficients):
    global last_result
    if "nc" not in _CACHE:
        _CACHE["nc"] = _build_nc()
    nc = _CACHE["nc"]
    in_maps = _host_prep(x, centers, coefficients)
    res = run_bass_kernel_spmd(nc, in_maps, core_ids=list(range(N_CORES)))
    last_result = res
    out = []
    for r in res.results:
        y = r["y"][:PER_CORE]
        out.append(y)
    return np.concatenate(out).astype(np.float32)


# revision 8
# speedup vs baseline: 1.6637x; 1.1594x over previous
"""Trainium2 Bass kernel for GaussianKernelLayer.

y[n] = sum_m softmax(coef)[m] * norm * exp(-0.5*|x_n - c_m|^2),
N=500000, M=256, D=4, sigma=1. Data-parallel over 8 cores (x sharded on N).

Device strategy (per core, NP=63488 padded points = 496 lanes... see below):

The exp work on the Scalar (ACT) engine is the hard floor: N*M/core =
16.25M elements at 1 elem/cycle/partition @ 1.2 GHz ~= 104 us. Everything
else is shaped to hide under that:

  - [point, center] layout: psum[pt, 256*a + ctr] holds the full exp
    argument z = x.c + ln(w*norm) - 0.5|c|^2 - 0.5|x|^2 for 8 point-blocks
    (a = 0..7) at once. Stationary = x-features [K=128, 128 pts] with the
    8 blocks STACKED along K (16 rows each: x_hi(4) x_hi(4) x_lo(4)
    sq_hi sq_lo 1 1); moving = a constant block-diagonal center matrix
    [128, 2048] so one 2048-column matmul computes 1024 points * 256
    centers (2 PE cycles/point, one instruction per 1024 points).
  - ACT does one big Exp per group: [128, 2048] PSUM f32 -> SBUF fp16.
  - DVE does a segmented reduce over centers: [128, 8, 256] -> [128, 8]
    fp16 (2x mode), writing y for 1024 points directly. No reduction
    matmuls, no partition reductions, no add passes.
  - All per-center and per-point bias terms are folded on the host into
    the fp16 hi/lo split streams; no device-side preamble, no DRAM
    bounce. DMA in ~2.5 MB, out 254 KB.

Point -> (lane, slot) mapping n = m*496 + (8g + a) keeps the final y DMA
partition-major (1984 B contiguous per partition).
"""

import math

import numpy as np

import concourse.bass as bass
import concourse.bacc as bacc_mod
import concourse.mybir as mybir
from concourse.bass_utils import run_bass_kernel_spmd
from concourse.tile import TileContext

N_CORES = 8
N_TOTAL = 500000
PER_CORE = N_TOTAL // N_CORES  # 62500
M = 256
D = 4
SIGMA = 1.0

NG = 62            # groups per core
BLK = 8            # point-blocks per group (stacked along K)
GPTS = 128 * BLK   # 1024 points per group
NP = NG * GPTS     # 63488 padded points per core
SLOTS = NP // 128  # 496 slots per lane
XCOLS = NG * 128   # 7936 stationary columns

F16 = mybir.dt.float16
F32 = mybir.dt.float32

_CACHE = {}


def _build_nc():
    nc = bacc_mod.Bacc()

    xs_d = nc.dram_tensor("xs", [128, XCOLS], F16, kind="ExternalInput")
    cd_d = nc.dram_tensor("cd", [128, 2 * M], F16, kind="ExternalInput")
    y_d = nc.dram_tensor("y", [NP], F32, kind="ExternalOutput")

    with TileContext(nc) as tc:
        with (
            tc.tile_pool(name="const", bufs=1) as constp,
            tc.tile_pool(name="xsp", bufs=8) as xsp,
            tc.tile_pool(name="expp", bufs=3) as expp,
            tc.tile_pool(name="redp", bufs=2) as redp,
            tc.tile_pool(name="yp", bufs=1) as yp,
            tc.tile_pool(name="psp", bufs=2, space="PSUM") as psp,
        ):
            cd_sb = constp.tile([128, 2 * M], F16)
            nc.sync.dma_start(cd_sb[:], cd_d[:])

            # stationary stream, 8 resident slabs of 8 groups each
            SLAB = 1024
            slabs = []
            for s in range(8):
                w = min(SLAB, XCOLS - s * SLAB)
                t = xsp.tile([128, SLAB], F16, tag="xs")
                nc.sync.dma_start(t[:, 0:w], xs_d[:, s * SLAB : s * SLAB + w])
                slabs.append(t)

            ys = yp.tile([128, SLOTS + 16], F16, tag="ys")

            for g in range(NG):
                s, off = divmod(g, 8)
                col = off * 128
                ps = psp.tile([128, BLK * M], F32, tag="ps")
                # ISA caps matmul moving free at 512: one matmul per pair of
                # point-blocks, PE 32-row tiles so the weight load is K=32.
                for a in range(4):
                    nc.tensor.matmul(
                        ps[:, 512 * a : 512 * (a + 1)],
                        slabs[s][32 * a : 32 * a + 32, col : col + 128],
                        cd_sb[32 * a : 32 * a + 32, :],
                        start=True,
                        stop=True,
                        tile_position=(32 * a, 0),
                    )
                ex = expp.tile([128, BLK * M], F16, tag="ex")
                nc.scalar.activation(
                    ex[:], ps[:], mybir.ActivationFunctionType.Exp
                )
                # tensor_reduce has no DVE 2x mode; halve with tensor_tensor
                # (2x/4x capable) first, reduce only the last 32 lanes.
                t1 = redp.tile([128, BLK * 128], F16, tag="t1")
                e3 = ex[:].rearrange("p (a c) -> p a c", c=M)
                h1 = t1[:].rearrange("p (a c) -> p a c", c=128)
                nc.vector.tensor_tensor(
                    h1, e3[:, :, 0:128], e3[:, :, 128:256], mybir.AluOpType.add
                )
                t2 = redp.tile([128, BLK * 64], F16, tag="t2")
                h2 = t2[:].rearrange("p (a c) -> p a c", c=64)
                nc.vector.tensor_tensor(
                    h2, h1[:, :, 0:64], h1[:, :, 64:128], mybir.AluOpType.add
                )
                t3 = redp.tile([128, BLK * 32], F16, tag="t3")
                h3 = t3[:].rearrange("p (a c) -> p a c", c=32)
                nc.vector.tensor_tensor(
                    h3, h2[:, :, 0:32], h2[:, :, 32:64], mybir.AluOpType.add
                )
                with nc.allow_low_precision(reason="fp16 y, rel tol 2e-2"):
                    nc.vector.tensor_reduce(
                        ys[:, BLK * g : BLK * (g + 1)],
                        h3,
                        axis=mybir.AxisListType.X,
                        op=mybir.AluOpType.add,
                    )
                # drain y in 4 overlapped chunks instead of one tail copy
                if g in (15, 31, 47, 61):
                    q = g // 16
                    c0 = 124 * q
                    yf = yp.tile([128, 124], F32, tag="yf")
                    nc.vector.tensor_copy(yf[:], ys[:, c0 : c0 + 124])
                    nc.sync.dma_start(
                        y_d.rearrange("(p f) -> p f", p=128)[:, c0 : c0 + 124],
                        yf[:],
                    )
    nc.compile()
    return nc


def _host_prep(x, centers, coefficients):
    """Host-side prep: softmax over 256 coefficients, fp16 hi/lo splits,
    per-center and per-point bias folding, streaming layout."""
    x = np.ascontiguousarray(np.asarray(x, dtype=np.float32))
    centers = np.asarray(centers, dtype=np.float32)
    coefficients = np.asarray(coefficients, dtype=np.float32)

    norm_const = np.float32(1.0 / ((2.0 * math.pi) ** (D / 2) * SIGMA**D))
    e = np.exp(coefficients - coefficients.max())
    w = (e / e.sum()).astype(np.float32)
    b = np.log(w * norm_const).astype(np.float32) - 0.5 * (centers**2).sum(axis=1)

    cT = centers.T  # [4, 256]
    c_hi = cT.astype(np.float16)
    c_lo = (cT - c_hi.astype(np.float32)).astype(np.float16)
    b_hi = b.astype(np.float16)
    b_lo = (b - b_hi.astype(np.float32)).astype(np.float16)

    crows = np.empty((16, M), dtype=np.float16)
    crows[0:4] = c_hi
    crows[4:8] = c_lo
    crows[8:12] = c_hi
    crows[12:14] = 1.0
    crows[14] = b_hi
    crows[15] = b_lo

    # [32, 512] two-block diagonal, replicated on all four 32-row bands so
    # band q's slice pairs with stationary rows 32q:32q+32 (blocks 2q, 2q+1)
    cd = np.zeros((128, 2 * M), dtype=np.float16)
    for q in range(4):
        cd[32 * q : 32 * q + 16, 0:M] = crows
        cd[32 * q + 16 : 32 * q + 32, M : 2 * M] = crows

    in_maps = []
    for i in range(N_CORES):
        xs = x[i * PER_CORE : (i + 1) * PER_CORE]
        xp = np.zeros((NP, D), dtype=np.float32)
        xp[:PER_CORE] = xs
        xh = xp.astype(np.float16)
        xl = (xp - xh.astype(np.float32)).astype(np.float16)
        sq = -0.5 * (xp * xp).sum(axis=1)
        sq_hi = sq.astype(np.float16)
        sq_lo = (sq - sq_hi.astype(np.float32)).astype(np.float16)

        feat = np.empty((16, NP), dtype=np.float16)
        feat[0:4] = xh.T      # pairs with c_hi
        feat[4:8] = xh.T      # pairs with c_lo
        feat[8:12] = xl.T     # pairs with c_hi
        feat[12] = sq_hi      # pairs with 1
        feat[13] = sq_lo      # pairs with 1
        feat[14:16] = 1.0     # pairs with b_hi / b_lo

        # n = m*496 + 8g + a  ->  xs_d[16a + k, g*128 + m] = feat[k, n]
        xsd = (
            feat.reshape(16, 128, NG, BLK)
            .transpose(3, 0, 2, 1)
            .reshape(128, XCOLS)
        )
        in_maps.append(
            {"xs": np.ascontiguousarray(xsd), "cd": cd.copy()}
        )
    return in_maps


last_result = None


def kernel(x, centers, coefficients):
    global last_result
    if "nc" not in _CACHE:
        _CACHE["nc"] = _build_nc()
    nc = _CACHE["nc"]
    in_maps = _host_prep(x, centers, coefficients)
    res = run_bass_kernel_spmd(nc, in_maps, core_ids=list(range(N_CORES)))
    last_result = res
    out = []
    for r in res.results:
        y = r["y"][:PER_CORE]
        out.append(y)
    return np.concatenate(out).astype(np.float32)


# revision 12
# speedup vs baseline: 1.6799x; 1.0098x over previous
"""Trainium2 Bass kernel for GaussianKernelLayer.

y[n] = sum_m softmax(coef)[m] * norm * exp(-0.5*|x_n - c_m|^2),
N=500000, M=256, D=4, sigma=1. Data-parallel over 8 cores (x sharded on N).

Device strategy (per core, NP=63488 padded points = 496 lanes... see below):

The exp work on the Scalar (ACT) engine is the hard floor: N*M/core =
16.25M elements at 1 elem/cycle/partition @ 1.2 GHz ~= 104 us. Everything
else is shaped to hide under that:

  - [point, center] layout: psum[pt, 256*a + ctr] holds the full exp
    argument z = x.c + ln(w*norm) - 0.5|c|^2 - 0.5|x|^2 for 8 point-blocks
    (a = 0..7) at once. Stationary = x-features [K=128, 128 pts] with the
    8 blocks STACKED along K (16 rows each: x_hi(4) x_hi(4) x_lo(4)
    sq_hi sq_lo 1 1); moving = a constant block-diagonal center matrix
    [128, 2048] so one 2048-column matmul computes 1024 points * 256
    centers (2 PE cycles/point, one instruction per 1024 points).
  - ACT does one big Exp per group: [128, 2048] PSUM f32 -> SBUF fp16.
  - DVE does a segmented reduce over centers: [128, 8, 256] -> [128, 8]
    fp16 (2x mode), writing y for 1024 points directly. No reduction
    matmuls, no partition reductions, no add passes.
  - All per-center and per-point bias terms are folded on the host into
    the fp16 hi/lo split streams; no device-side preamble, no DRAM
    bounce. DMA in ~2.5 MB, out 254 KB.

Point -> (lane, slot) mapping n = m*496 + (8g + a) keeps the final y DMA
partition-major (1984 B contiguous per partition).
"""

import math

import numpy as np

import concourse.bass as bass
import concourse.bacc as bacc_mod
import concourse.mybir as mybir
from concourse.bass_utils import run_bass_kernel_spmd
from concourse.tile import TileContext

N_CORES = 8
N_TOTAL = 500000
PER_CORE = N_TOTAL // N_CORES  # 62500
M = 256
D = 4
SIGMA = 1.0

NG = 62            # groups per core
BLK = 8            # point-blocks per group (stacked along K)
GPTS = 128 * BLK   # 1024 points per group
NP = NG * GPTS     # 63488 padded points per core
SLOTS = NP // 128  # 496 slots per lane
XCOLS = NG * 128   # 7936 stationary columns

F16 = mybir.dt.float16
F32 = mybir.dt.float32

_CACHE = {}


def _build_nc():
    nc = bacc_mod.Bacc()

    xs_d = nc.dram_tensor("xs", [128, XCOLS], F16, kind="ExternalInput")
    cd_d = nc.dram_tensor("cd", [128, 2 * M], F16, kind="ExternalInput")
    y_d = nc.dram_tensor("y", [NP], F32, kind="ExternalOutput")

    with TileContext(nc) as tc:
        with (
            tc.tile_pool(name="const", bufs=1) as constp,
            tc.tile_pool(name="xsp", bufs=8) as xsp,
            tc.tile_pool(name="expp", bufs=3) as expp,
            tc.tile_pool(name="redp", bufs=2) as redp,
            tc.tile_pool(name="yp", bufs=1) as yp,
            tc.tile_pool(name="psp", bufs=2, space="PSUM") as psp,
        ):
            cd_sb = constp.tile([128, 2 * M], F16)
            nc.sync.dma_start(cd_sb[:], cd_d[:])

            # stationary stream: small first slab so matmul 0 starts early,
            # then 1024-col slabs; all resident (15.5 KB/partition)
            widths = [128, 896] + [1024] * 6 + [768]
            starts = [0]
            for w in widths[:-1]:
                starts.append(starts[-1] + w)
            slabs = []
            for s, (c0, w) in enumerate(zip(starts, widths)):
                t = xsp.tile([128, w], F16, tag=f"xs{s}", bufs=1, name=f"xs{s}")
                nc.sync.dma_start(t[:], xs_d[:, c0 : c0 + w])
                slabs.append(t)

            def slab_of(g):
                c = 128 * g
                for s in range(len(starts) - 1, -1, -1):
                    if c >= starts[s]:
                        return s, c - starts[s]
                raise AssertionError

            ys = yp.tile([128, SLOTS], F32, tag="ys")

            for g in range(NG):
                s, col = slab_of(g)
                ps = psp.tile([128, BLK * M], F32, tag="ps")
                # ISA caps matmul moving free at 512: one matmul per pair of
                # point-blocks, PE 32-row tiles so the weight load is K=32.
                for a in range(4):
                    nc.tensor.matmul(
                        ps[:, 512 * a : 512 * (a + 1)],
                        slabs[s][32 * a : 32 * a + 32, col : col + 128],
                        cd_sb[32 * a : 32 * a + 32, :],
                        start=True,
                        stop=True,
                        tile_position=(32 * a, 0),
                    )
                ex = expp.tile([128, BLK * M], F16, tag="ex")
                nc.scalar.activation(
                    ex[:], ps[:], mybir.ActivationFunctionType.Exp
                )
                # tensor_reduce has no DVE 2x mode; halve with tensor_tensor
                # (2x/4x capable) first, reduce only the last 32 lanes.
                t1 = redp.tile([128, BLK * 128], F16, tag="t1")
                e3 = ex[:].rearrange("p (a c) -> p a c", c=M)
                h1 = t1[:].rearrange("p (a c) -> p a c", c=128)
                nc.vector.tensor_tensor(
                    h1, e3[:, :, 0:128], e3[:, :, 128:256], mybir.AluOpType.add
                )
                t2 = redp.tile([128, BLK * 64], F16, tag="t2")
                h2 = t2[:].rearrange("p (a c) -> p a c", c=64)
                nc.vector.tensor_tensor(
                    h2, h1[:, :, 0:64], h1[:, :, 64:128], mybir.AluOpType.add
                )
                t3 = redp.tile([128, BLK * 32], F16, tag="t3")
                h3 = t3[:].rearrange("p (a c) -> p a c", c=32)
                nc.vector.tensor_tensor(
                    h3, h2[:, :, 0:32], h2[:, :, 32:64], mybir.AluOpType.add
                )
                nc.vector.tensor_reduce(
                    ys[:, BLK * g : BLK * (g + 1)],
                    h3,
                    axis=mybir.AxisListType.X,
                    op=mybir.AluOpType.add,
                )
                # drain y directly from ys in overlapped chunks; the final
                # chunk is tiny so the post-loop tail is short
                drains = {15: (0, 124), 31: (124, 248), 47: (248, 372),
                          59: (372, 480), 61: (480, 496)}
                if g in drains:
                    c0, c1 = drains[g]
                    nc.sync.dma_start(
                        y_d.rearrange("(p f) -> p f", p=128)[:, c0:c1],
                        ys[:, c0:c1],
                    )
    nc.compile()
    return nc


def _host_prep(x, centers, coefficients):
    """Host-side prep: softmax over 256 coefficients, fp16 hi/lo splits,
    per-center and per-point bias folding, streaming layout."""
    x = np.ascontiguousarray(np.asarray(x, dtype=np.float32))
    centers = np.asarray(centers, dtype=np.float32)
    coefficients = np.asarray(coefficients, dtype=np.float32)

    norm_const = np.float32(1.0 / ((2.0 * math.pi) ** (D / 2) * SIGMA**D))
    e = np.exp(coefficients - coefficients.max())
    w = (e / e.sum()).astype(np.float32)
    b = np.log(w * norm_const).astype(np.float32) - 0.5 * (centers**2).sum(axis=1)

    cT = centers.T  # [4, 256]
    c_hi = cT.astype(np.float16)
    c_lo = (cT - c_hi.astype(np.float32)).astype(np.float16)
    b_hi = b.astype(np.float16)
    b_lo = (b - b_hi.astype(np.float32)).astype(np.float16)

    crows = np.empty((16, M), dtype=np.float16)
    crows[0:4] = c_hi
    crows[4:8] = c_lo
    crows[8:12] = c_hi
    crows[12:14] = 1.0
    crows[14] = b_hi
    crows[15] = b_lo

    # [32, 512] two-block diagonal, replicated on all four 32-row bands so
    # band q's slice pairs with stationary rows 32q:32q+32 (blocks 2q, 2q+1)
    cd = np.zeros((128, 2 * M), dtype=np.float16)
    for q in range(4):
        cd[32 * q : 32 * q + 16, 0:M] = crows
        cd[32 * q + 16 : 32 * q + 32, M : 2 * M] = crows

    in_maps = []
    for i in range(N_CORES):
        xs = x[i * PER_CORE : (i + 1) * PER_CORE]
        xp = np.zeros((NP, D), dtype=np.float32)
        xp[:PER_CORE] = xs
        xh = xp.astype(np.float16)
        xl = (xp - xh.astype(np.float32)).astype(np.float16)
        sq = -0.5 * (xp * xp).sum(axis=1)
        sq_hi = sq.astype(np.float16)
        sq_lo = (sq - sq_hi.astype(np.float32)).astype(np.float16)

        feat = np.empty((16, NP), dtype=np.float16)
        feat[0:4] = xh.T      # pairs with c_hi
        feat[4:8] = xh.T      # pairs with c_lo
        feat[8:12] = xl.T     # pairs with c_hi
        feat[12] = sq_hi      # pairs with 1
        feat[13] = sq_lo      # pairs with 1
        feat[14:16] = 1.0     # pairs with b_hi / b_lo

        # n = m*496 + 8g + a  ->  xs_d[16a + k, g*128 + m] = feat[k, n]
        xsd = (
            feat.reshape(16, 128, NG, BLK)
            .transpose(3, 0, 2, 1)
            .reshape(128, XCOLS)
        )
        in_maps.append(
            {"xs": np.ascontiguousarray(xsd), "cd": cd.copy()}
        )
    return in_maps


last_result = None


def kernel(x, centers, coefficients):
    global last_result
    if "nc" not in _CACHE:
        _CACHE["nc"] = _build_nc()
    nc = _CACHE["nc"]
    in_maps = _host_prep(x, centers, coefficients)
    res = run_bass_kernel_spmd(nc, in_maps, core_ids=list(range(N_CORES)))
    last_result = res
    out = []
    for r in res.results:
        y = r["y"][:PER_CORE]
        out.append(y)
    return np.concatenate(out).astype(np.float32)
